# revision 29
# baseline (speedup 1.0000x reference)
# Dynamic convolution (CondConv-style) Trainium2 Bass kernel, v4.
#
# Problem: x [16, 128, 128, 128]; per-sample attention over K=4 expert
# 3x3 conv kernels; per-sample aggregated conv + bias.
#
# Strategy: data-parallel over batch, 2 samples per core on 8 cores.
# Key structure (vs the v1 baseline):
#   - Expert banks (bf16) + attention weights are pre-transposed on the
#     host; no device-side transpose prologue at all.
#   - x is DMA'd straight into the zero-haloed f32r tile (512B lines,
#     full DMA rate) — no staging pass. The pooled mean comes from flat
#     contiguous per-chunk DVE reduces spanning the zeroed halo columns.
#   - Conv in f32r (self-loading matmuls, 1 cycle/row at N=512; bf16
#     would add 576 legalization-inserted Ldweights on the PE SEQ).
#   - Output drains to bf16 (host upcasts), halving store traffic.
#   - dma_start's semaphore waits block the issuing engine's SEQ, so
#     output stores issue from the otherwise-idle GPSIMD (Pool) queue;
#     x loads own the SP queue. Neither blocks drains or loads.
#   - Software pipelining: sample b's x loads + pooled reduces run
#     during sample b-1's conv; b's attention+mixing instruction stream
#     is spliced into the middle of b-1's conv chunk loop, so the PE
#     reaches b's conv with its mixed weights already in SBUF — the PE
#     never idles between conv streams, keeping its p-state ramped.
#
# Measured cycle model (per N=512 matmul): 512 compute + ~32 deduped
# stationary loads + ~62 residual load/bank-pipeline overhead = 606 cyc
# -> ~146us/rep, with the PE 100% busy. Known next step if revisited:
# 1D Winograd F(2,3) along width (host G-transforms the expert banks to
# 12 position-weights before the same on-device mixing; V-transform and
# output-transform are cheap byte-limited DVE ops) cuts PE flops 1.5x
# for a projected ~112us/rep, at the cost of PSUM sub-bank packing and
# stride-2 output writes.
import os

import numpy as np

B, C, H, W = 16, 128, 128, 128
K, HID, KS = 4, 64, 3
TEMP = 30.0
N_CORES = 8
BPC = B // N_CORES  # samples per core
HP, WP = H + 2, W + 2  # padded spatial
QROWS = int(os.environ.get("KERNEL_QROWS", "8"))  # x load chunk height
NQ = H // QROWS
RPC = 4  # conv rows per PSUM chunk ([C, RPC*W] fp32 = one PSUM bank)
NCHUNK = H // RPC
TAPS = KS * KS
# conv chunks per megagroup: 4 with a 7-buf PSUM pool means a new
# group's banks were drained ~2 groups ago (huge recycle margin), while
# 7 would reuse banks just-in-time and stall on HW drain+sem latency
MEGA = int(os.environ.get("KERNEL_MEGA", "4"))
# megagroup boundaries: e.g. MEGA=7 -> [7, 7, 7, 7, 4]
_g, GROUPS = 0, []
while _g < NCHUNK:
    GROUPS.append((_g, min(_g + MEGA, NCHUNK)))
    _g += MEGA
ATT_AT = int(os.environ.get("KERNEL_ATT_AT", "22"))  # splice point

_cache = {}


def _enable_ldw_opt():
    """Flip walrus's --enable-ldw-opt to true for compiles from this
    process: the tap-major conv emits runs of self-loading matmuls that
    share their stationary weights, and this pass elides the redundant
    reloads (each reload costs un-overlapped PE cycles, a ~25% tax on
    N=512 matmuls)."""
    import concourse.bass_utils as _bu

    if getattr(_bu, "_ldw_opt_patched", False):
        return
    _orig = _bu.run_command

    def _patched(cmd, *a, **k):
        if isinstance(cmd, list):
            cmd = [
                "--enable-ldw-opt=true" if c == "--enable-ldw-opt=false" else c
                for c in cmd
            ]
        return _orig(cmd, *a, **k)

    _bu.run_command = _patched
    _bu._ldw_opt_patched = True


def _build(repeat: int = 1):
    """Build + compile the Bass program (same program for all 8 cores)."""
    if os.environ.get("KERNEL_LDW", "1") == "1":
        _enable_ldw_opt()
    from contextlib import ExitStack

    import concourse.bacc as bacc
    import concourse.mybir as mybir
    import concourse.tile as tile

    fp32 = mybir.dt.float32
    f32r = mybir.dt.float32r
    bf16 = mybir.dt.bfloat16
    AF = mybir.ActivationFunctionType
    AX = mybir.AxisListType
    ALU = mybir.AluOpType

    nc = bacc.Bacc(
        "TRN2",
        target_bir_lowering=False,
        debug=False,
        enable_asserts=False,
        num_devices=N_CORES,
    )

    x_d = nc.dram_tensor("x", (BPC, C, H, W), f32r, kind="ExternalInput").ap()
    wTe_d = nc.dram_tensor("wTe", (K, C, TAPS * C), bf16, kind="ExternalInput").ap()
    w1t_d = nc.dram_tensor("w1t", (C, HID), fp32, kind="ExternalInput").ap()
    w2t_d = nc.dram_tensor("w2t", (HID, K), fp32, kind="ExternalInput").ap()
    biast_d = nc.dram_tensor("biast", (C, K), fp32, kind="ExternalInput").ap()
    out_d = nc.dram_tensor("out", (BPC, C, H, W), bf16, kind="ExternalOutput").ap()
    out_flat = out_d.rearrange("b c h w -> b c (h w)")

    with tile.TileContext(nc) as tc, ExitStack() as ctx:
        consts = ctx.enter_context(tc.tile_pool(name="consts", bufs=1))
        wpool = ctx.enter_context(tc.tile_pool(name="wpool", bufs=1))
        mixp = ctx.enter_context(tc.tile_pool(name="mixp", bufs=1))
        xpool = ctx.enter_context(tc.tile_pool(name="xpool", bufs=2))
        smalls = ctx.enter_context(tc.tile_pool(name="smalls", bufs=4))
        stage = ctx.enter_context(tc.tile_pool(name="stage", bufs=3))
        wTpool = ctx.enter_context(tc.tile_pool(name="wTp", bufs=2))
        cpsum = ctx.enter_context(tc.tile_pool(name="cpsum", bufs=7, space="PSUM"))
        spsum = ctx.enter_context(tc.tile_pool(name="spsum", bufs=1, space="PSUM"))

        # ---- constants / weights (one-time, outside the rep loop) ----
        ones_row = consts.tile([1, C], fp32, name="ones_row")
        nc.vector.memset(ones_row, 1.0)
        zero_col = consts.tile([C, HP], fp32, name="zero_col")
        nc.vector.memset(zero_col, 0.0)

        wTe = [wpool.tile([C, TAPS, C], bf16, name=f"wTe{k}") for k in range(K)]
        for k in range(K):
            nc.sync.dma_start(
                out=wTe[k], in_=wTe_d[k].rearrange("c (s o) -> c s o", s=TAPS)
            )
        wTe_flat = [t.rearrange("c s o -> c (s o)") for t in wTe]
        w1T = wpool.tile([C, HID], fp32, name="w1T")
        nc.sync.dma_start(out=w1T, in_=w1t_d)
        w2T = wpool.tile([HID, K], fp32, name="w2T")
        nc.sync.dma_start(out=w2T, in_=w2t_d)
        biasT = wpool.tile([C, K], fp32, name="biasT")
        nc.sync.dma_start(out=biasT, in_=biast_d)

        acc_t = mixp.tile([C, TAPS * C], fp32, name="acc_t")

        def phase_load(b):
            """Borders + x DMAs + pooled partial reduces for sample b."""
            x_pad = xpool.tile([C, HP, WP], f32r, name="x_pad")
            x_flat = x_pad.rearrange("c h w -> c (h w)")
            # zero halo: top/bottom rows + left/right columns (an f32r
            # memset fails the walrus ISA check; copy from an fp32 zero
            # const instead, as v1 did)
            nc.vector.tensor_copy(x_pad[:, 0, :], zero_col)
            nc.vector.tensor_copy(x_pad[:, HP - 1, :], zero_col)
            nc.vector.tensor_copy(x_pad[:, :, 0], zero_col)
            nc.vector.tensor_copy(x_pad[:, :, WP - 1], zero_col)

            partials = smalls.tile([C, NQ], fp32, name="partials")
            for q in range(NQ):
                # straight into the haloed tile: 512B contiguous lines
                nc.sync.dma_start(
                    out=x_pad[:, 1 + q * QROWS : 1 + (q + 1) * QROWS, 1 : W + 1],
                    in_=x_d[b, :, q * QROWS : (q + 1) * QROWS, :],
                )
                # pooled-mean partial: flat contiguous rows; halo columns
                # are already zero so they don't perturb the sum
                nc.vector.reduce_sum(
                    out=partials[:, q : q + 1],
                    in_=x_flat[
                        :, (1 + q * QROWS) * WP : (1 + (q + 1) * QROWS) * WP
                    ],
                    axis=AX.X,
                )
            return x_pad, partials

        def phase_att(b, lstate):
            """Attention MLP + softmax + bias/weight mixing for sample b.

            Spliced into the middle of the previous sample's conv chunk
            loop: by then the pooled sum is ready, so the PE matmuls here
            retire immediately and the PE p-state stays ramped.
            """
            x_pad, partials = lstate
            pooled = smalls.tile([C, 1], fp32, name="pooled")
            nc.vector.reduce_sum(out=pooled, in_=partials, axis=AX.X)

            # attention MLP (w1T pre-scaled by 1/(H*W) on the host)
            h_ps = spsum.tile([HID, 1], fp32, name="h_ps", tag="sps")
            nc.tensor.matmul(h_ps, w1T, pooled, start=True, stop=True)
            h_sb = smalls.tile([HID, 1], fp32, name="h_sb")
            nc.scalar.activation(h_sb, h_ps, AF.Relu)

            log_ps = spsum.tile([1, K], fp32, name="log_ps", tag="sps")
            nc.tensor.matmul(log_ps, h_sb, w2T, start=True, stop=True)

            # softmax: logits/TEMP are tiny (pooled means of unit
            # gaussians), so skip max-subtraction; exp + sum in one op
            att_e = smalls.tile([1, K], fp32, name="att_e")
            esum = smalls.tile([1, 1], fp32, name="esum")
            nc.scalar.activation(
                att_e, log_ps, AF.Exp, scale=1.0 / TEMP, accum_out=esum
            )
            rsum = smalls.tile([1, 1], fp32, name="rsum")
            nc.vector.reciprocal(rsum, esum)
            att_row = smalls.tile([1, K], fp32, name="att_row")
            nc.vector.tensor_scalar_mul(att_row, att_e, rsum)

            # broadcast normalized att to all partitions via K=1 matmul
            attb_ps = spsum.tile([C, K], fp32, name="attb_ps", tag="sps")
            nc.tensor.matmul(attb_ps, ones_row, att_row, start=True, stop=True)
            att_bc = smalls.tile([C, K], fp32, name="att_bc")
            nc.vector.tensor_copy(att_bc, attb_ps)

            # aggregated bias [C, 1]
            btmp = smalls.tile([C, K], fp32, name="btmp")
            nc.vector.tensor_mul(btmp, biasT, att_bc)
            aggb = smalls.tile([C, 1], fp32, name="aggb")
            nc.vector.reduce_sum(out=aggb, in_=btmp, axis=AX.X)

            # expert mixing -> per-sample conv weights (4 fused DVE ops);
            # the final write provides the f32r rounding (mixed
            # f32r/bf16 matmul operands are rejected by the verifier)
            wT = wTpool.tile([C, TAPS, C], f32r, name="wT")
            wT_f = wT.rearrange("c s o -> c (s o)")
            nc.vector.tensor_scalar_mul(acc_t, wTe_flat[0], att_bc[:, 0:1])
            for kk in (1, 2):
                nc.vector.scalar_tensor_tensor(
                    acc_t,
                    wTe_flat[kk],
                    att_bc[:, kk : kk + 1],
                    acc_t,
                    op0=ALU.mult,
                    op1=ALU.add,
                )
            nc.vector.scalar_tensor_tensor(
                wT_f, wTe_flat[3], att_bc[:, 3:4], acc_t, op0=ALU.mult, op1=ALU.add
            )
            return x_pad, wT, aggb

        def phase_conv(b, state, mid=None):
            """Conv + bias + store for sample b; `mid()` is spliced in
            mid-loop (the next sample's attention+mixing).

            Tap-major within megagroups of up to MEGA chunks: the
            matmuls of one tap share their stationary weights, so with
            walrus's redundant-load-weight optimization only 9 weight
            loads remain per megagroup. Tap-major also drains bank j
            during the tail of the last tap row, so bank recycling is
            just-in-time even at 7 live banks."""
            x_pad, wT, aggb = state
            for c0, c1 in GROUPS:
                ng = c1 - c0
                og = stage.tile([C, MEGA * RPC * W], bf16, name="og")
                if mid is not None and c0 <= ATT_AT < c1:
                    phase_conv.mid_result = mid()
                cps = [
                    cpsum.tile([C, RPC * W], fp32, name="cps") for _ in range(ng)
                ]
                # forward tap-major order: consecutive matmuls alternate
                # PSUM banks (same-bank back-to-back stalls on the
                # accumulate read-modify-write; serpentine order measured
                # 29us slower), while same-tap runs let walrus's ldw-opt
                # elide redundant stationary loads
                for s in range(TAPS):
                    dy, dx = s // KS, s % KS
                    for cc in range(ng):
                        h0 = (c0 + cc) * RPC
                        rhs = x_pad[:, h0 + dy : h0 + dy + RPC, dx : dx + W]
                        nc.tensor.matmul(
                            cps[cc],
                            wT[:, s, :],
                            rhs,
                            start=(s == 0),
                            stop=(s == TAPS - 1),
                        )
                split = os.environ.get("KERNEL_SPLITDRAIN", "1") == "1"
                for cc in range(ng):
                    dst = og[:, cc * RPC * W : (cc + 1) * RPC * W]
                    if cc % 2 == 0 or not split:
                        nc.scalar.activation(
                            dst, cps[cc], AF.Identity, bias=aggb, scale=1.0
                        )
                    else:
                        # alternate drains onto DVE: doubles drain-burst
                        # throughput so bank recycling keeps up with the
                        # matmul tail even at 7 live banks
                        nc.vector.tensor_scalar_add(dst, cps[cc], aggb)
                # store from the idle GPSIMD queue: its SEQ blocking on
                # the drain semaphores is harmless there
                nc.gpsimd.dma_start(
                    out=out_flat[b, :, c0 * RPC * W : c1 * RPC * W],
                    in_=og[:, : ng * RPC * W],
                )

        # ---- software-pipelined schedule over (rep, sample) ----
        steps = [(r, b) for r in range(repeat) for b in range(BPC)]
        state = phase_att(0, phase_load(0))
        for i, (r, b) in enumerate(steps):
            if i + 1 < len(steps):
                nxt_load = phase_load(steps[i + 1][1])
                mid = lambda nl=nxt_load, nb=steps[i + 1][1]: phase_att(nb, nl)
                phase_conv(b, state, mid=mid)
                state = phase_conv.mid_result
            else:
                phase_conv(b, state)

    nc.compile()
    return nc


def _get_prog():
    repeat = int(os.environ.get("KERNEL_REPEAT", "1"))
    if repeat not in _cache:
        _cache[repeat] = _build(repeat)
    return _cache[repeat]


def _prep_inputs(x, att_w1, att_w2, weight, bias):
    import ml_dtypes

    x = np.asarray(x, dtype=np.float32)
    # expert banks -> [k, ci, tap*co], bf16
    wTe = np.ascontiguousarray(
        np.transpose(np.asarray(weight, dtype=np.float32), (0, 2, 3, 4, 1)).reshape(
            K, C, TAPS * C
        )
    ).astype(ml_dtypes.bfloat16)
    # attention weights pre-transposed; w1 also absorbs the 1/(H*W) mean
    w1t = np.ascontiguousarray(np.asarray(att_w1, dtype=np.float32).T) / (H * W)
    w1t = w1t.astype(np.float32)
    w2t = np.ascontiguousarray(np.asarray(att_w2, dtype=np.float32).T)
    biast = np.ascontiguousarray(np.asarray(bias, dtype=np.float32).T)
    in_maps = []
    for i in range(N_CORES):
        in_maps.append(
            {
                "x": np.ascontiguousarray(x[i * BPC : (i + 1) * BPC]),
                "wTe": wTe,
                "w1t": w1t,
                "w2t": w2t,
                "biast": biast,
            }
        )
    return in_maps


def kernel(x, att_w1, att_w2, weight, bias):
    from concourse.bass_utils import run_bass_kernel_spmd

    nc = _get_prog()
    in_maps = _prep_inputs(x, att_w1, att_w2, weight, bias)
    res = run_bass_kernel_spmd(nc, in_maps, list(range(N_CORES)))
    kernel.last_results = res
    return np.concatenate(
        [r["out"].astype(np.float32) for r in res.results], axis=0
    )
